# revision 1
# baseline (speedup 1.0000x reference)
"""Trainium2 Bass kernel: masked-bilinear channel-mixing Conv2d.

reference math (N=4, C=96, H=W=32, O=96, K=3, PAD=1):
    p = avgpool3x3(x, count_include_pad) -> [N, C, H, W] -> [N, L=1024, C]
    wm = weight * mask                              [O, C, C]
    y[n,l,o] = sum_{c,d} wm[o,c,d] p[n,l,c] p[n,l,d] + bias[o]

Sharding: data-parallel over the 4096 spatial locations -> 8 cores, each
takes half of one image (16 rows = 512 locations) and computes all 96
output channels. Weight/mask are replicated (host pre-transposes to
c-major, block-interleaved, so each block arrives in one contiguous DMA);
the avg-pool 1/9 scale and the weight*mask product run on device.

Per-core device pipeline (bf16 matmul operands, f32 PSUM accumulate):
  warmup: dummy matmul burst during the DMA/pool phase (the PE clock gate
          takes ~16us of sustained activity to lift 1.2GHz -> 2.4GHz)
  pooling (4 DVE adds + scale)        -> pt16 [96(c), 512(loc)] bf16
  wm16 = wt * mt                      -> [96(c), 9216(o,d)] bf16 (GPSIMD)
  o-loop, triples (3m, 3m+1, 3m+2):
    T_o  = matmul(lhsT=wm16[:, o], rhs=pt16)     # [96(d), 512] f32 PSUM
    z    = (T ⊙ pt16) batched per triple, routed to one of:
             direct DVE TT (PSUM src) | ACT copy + DVE bf16 2x TT |
             ACT copy + GPSIMD TT
    y[r(o)] = matmul(lhsT=onehot32, rhs=z_o)     # M=32; r(o)=32*(o%3)+o//3
             consecutive o's hit PE column-groups 0/1/2 -> 3x concurrent,
             and a triple shares one lhsT (same one-hot column m=o//3)
  y_sb = y + bias_perm; DMA out un-permutes rows via a reordered DRAM AP.
"""
import numpy as np

import concourse.bass as bass
import concourse.bacc as bacc
import concourse.mybir as mybir
from concourse import tile
from concourse import bass_utils

C = 96
O = 96
HS = 16           # rows per core shard
W = 32
L = HS * W        # 512 locations per core
N_CORES = 8
NBLK = 8          # weight/mask DMA + multiply blocks
WARMUP_MMS = 10
KEEPWARM_FROM = 99     # disabled
KEEPWARM_PER_TRIPLE = 0
F32 = mybir.dt.float32
BF16 = mybir.dt.bfloat16

# z-route per o-triple (m = o//3): R3 via ACT copy + GPSIMD TT,
# R1 direct DVE TT from PSUM, R2 (default) ACT copy + DVE bf16 2x TT.
R3_TRIPLES = set()
R1_TRIPLES = set()
WM_DVE_BLOCKS = {0, 1}   # rest go to GPSIMD


def _build_kernel(nc: bass.Bass):
    xs_d = nc.dram_tensor("xs", [C, 18 * 34], F32, kind="ExternalInput")
    wmcat_d = nc.dram_tensor("wmcat", [C, 2 * O * C], F32, kind="ExternalInput")
    b_d = nc.dram_tensor("bias", [128, 3], F32, kind="ExternalInput")
    y_d = nc.dram_tensor("y", [O, L], F32, kind="ExternalOutput")

    with tile.TileContext(nc) as tc:
        with (
            tc.tile_pool(name="const", bufs=1) as cpool,
            tc.tile_pool(name="work", bufs=1) as wpool,
            tc.tile_pool(name="tc3", bufs=6) as tcpool,
            tc.tile_pool(name="z", bufs=6) as zpool,
            tc.tile_pool(name="tpsum", bufs=2, space="PSUM") as tpsum,
            tc.tile_pool(name="ypsum", bufs=1, space="PSUM") as ypsum,
            tc.tile_pool(name="wpsum", bufs=1, space="PSUM") as wpsum,
        ):
            xs = cpool.tile([C, 18 * 34], F32)
            wmcat = cpool.tile([C, 2 * O * C], F32)
            wm16 = cpool.tile([C, O * C], BF16)
            bias = cpool.tile([128, 3], F32)
            # zob[:, 31] is ones, else zero; zob[:, 31-m : 63-m] is [96, 32]
            # with ones in column m -> as lhsT it scatters the partition-sum
            # of rhs into row (32*colgroup + m) of the output.
            zob = cpool.tile([C, 63], BF16)
            warm16 = cpool.tile([C, L], BF16)
            nc.sync.dma_start(xs[:], xs_d.ap())
            nc.sync.dma_start(bias[:], b_d.ap())
            nc.vector.memset(warm16[:], 0.0)

            # PE warmup: garbage matmuls while DMA/pool/wm stages run
            wps = wpsum.tile([C, L], F32)
            for _ in range(WARMUP_MMS):
                nc.tensor.matmul(wps[:], warm16[:, 0:C], warm16[:],
                                 start=True, stop=True, skip_group_check=True)

            # weight/mask: host packs [blk] = [wt_blk | mt_blk] so each wm
            # block waits on exactly one DMA-queue semaphore; multiplies
            # overlap the transfers.
            BL = O * C // NBLK
            for blk in range(NBLK):
                base = blk * 2 * BL
                nc.sync.dma_start(wmcat[:, base:base + 2 * BL],
                                  wmcat_d.ap()[:, base:base + 2 * BL])

            def wm_block(blk):
                base = blk * 2 * BL
                eng = nc.vector if blk < 2 else nc.gpsimd
                eng.tensor_mul(
                    wm16[:, blk * BL:(blk + 1) * BL],
                    wmcat[:, base:base + BL],
                    wmcat[:, base + BL:base + 2 * BL])

            # --- pooling: horizontal then vertical 3-tap box sums ---
            # (emitted BEFORE the wm multiplies: pooling gates the whole
            # matmul pipeline, the wm blocks only gate their own o-range)
            s1 = wpool.tile([C, 18 * 32], F32)
            s2 = wpool.tile([C, 18 * 32], F32)
            pt_raw = wpool.tile([C, L], F32)
            pt16 = wpool.tile([C, L], BF16)
            pt2 = wpool.tile([C, L], F32)
            x3 = xs[:].rearrange("c (h w) -> c h w", h=18)
            s1v = s1[:].rearrange("c (h w) -> c h w", h=18)
            s2v = s2[:].rearrange("c (h w) -> c h w", h=18)
            nc.vector.tensor_add(s1v, x3[:, :, 0:32], x3[:, :, 1:33])
            nc.vector.tensor_add(s2v, s1v, x3[:, :, 2:34])
            ptv = pt_raw[:].rearrange("c (h w) -> c h w", h=HS)
            pt2v = pt2[:].rearrange("c (h w) -> c h w", h=HS)
            nc.vector.tensor_add(pt2v, s2v[:, 0:16, :], s2v[:, 1:17, :])
            nc.vector.tensor_add(ptv, pt2v, s2v[:, 2:18, :])
            # p = boxsum/9; p enters the quadratic form twice -> 1/81 total
            nc.vector.tensor_scalar_mul(pt16[:], pt_raw[:], 1.0 / 9.0)
            wm_block(0)
            wm_block(1)
            nc.vector.memset(zob[:], 0.0)
            nc.vector.memset(zob[:, 31:32], 1.0)

            # pass-1 matmuls are split into 3 explicit M=32 column-tiles
            # (PE col groups 0-2); ALL pass-2 reductions live on col group 3
            # (y rows 96-127, one-hot column o%32), software-pipelined one
            # triple behind pass-1 -> the two streams run concurrently on
            # disjoint column groups.
            y_ps = ypsum.tile([128, L], F32)
            y_stage = wpool.tile([128, 3 * L], F32)
            zq = []          # pending (o, z3, k) for pass-2

            def pass2_flush(keep=0):
                while len(zq) > keep:
                    o, z3t, k = zq.pop(0)
                    nc.tensor.matmul(
                        y_ps[96:128, :],
                        zob[:, 31 - (o % 32):63 - (o % 32)],
                        z3t[:, k * L:(k + 1) * L],
                        start=(o % 32 == 0), stop=(o % 32 == 31),
                        skip_group_check=True, tile_position=(0, 96),
                    )
                    if o % 32 == 31:
                        b = o // 32
                        # stage the finished 32-channel block out of PSUM
                        # (with bias) so the next block can restart the bank,
                        # and ship it to DRAM immediately
                        nc.vector.tensor_scalar_add(
                            y_stage[96:128, b * L:(b + 1) * L],
                            y_ps[96:128, :], bias[96:128, b:b + 1])
                        nc.sync.dma_start(
                            y_d.ap()[32 * b:32 * (b + 1), :],
                            y_stage[96:128, b * L:(b + 1) * L])

            for m in range(32):
                if m % 4 == 0 and m // 4 + 2 < NBLK:
                    wm_block(m // 4 + 2)
                os3 = (3 * m, 3 * m + 1, 3 * m + 2)
                t3 = tpsum.tile([C, 3 * L], F32)
                for k, o in enumerate(os3):
                    for s in range(3):
                        nc.tensor.matmul(
                            t3[32 * s:32 * (s + 1), k * L:(k + 1) * L],
                            wm16[:, o * C + 32 * s:o * C + 32 * (s + 1)],
                            pt16[:], start=True, stop=True,
                        )
                pass2_flush(keep=6)
                z3 = zpool.tile([C, 3 * L], BF16, tag="z3")
                pt3 = pt16[:].unsqueeze(1).broadcast_to((C, 3, L))
                tc3 = tcpool.tile([C, 3 * L], BF16, tag="tc3")
                nc.scalar.activation(
                    tc3[:], t3[:], mybir.ActivationFunctionType.Copy)
                nc.vector.tensor_mul(
                    z3[:].rearrange("c (k l) -> c k l", k=3),
                    tc3[:].rearrange("c (k l) -> c k l", k=3), pt3)
                for k, o in enumerate(os3):
                    zq.append((o, z3, k))
            pass2_flush()


    return nc


_NC_CACHE = {}


def _get_nc():
    if "nc" not in _NC_CACHE:
        nc = bacc.Bacc("TRN2", target_bir_lowering=False, debug=False,
                       enable_asserts=False)
        _build_kernel(nc)
        nc.compile()
        _NC_CACHE["nc"] = nc
    return _NC_CACHE["nc"]


def _prep_shards(x, weight, mask, bias):
    xpad = np.pad(np.asarray(x, np.float32), ((0, 0), (0, 0), (1, 1), (1, 1)))
    wt = np.asarray(weight, np.float32).transpose(1, 0, 2).reshape(C, O * C)
    mt = np.asarray(mask, np.float32).transpose(1, 0, 2).reshape(C, O * C)
    BL = O * C // NBLK
    wmcat = np.empty((C, NBLK, 2, BL), np.float32)
    wmcat[:, :, 0, :] = wt.reshape(C, NBLK, BL)
    wmcat[:, :, 1, :] = mt.reshape(C, NBLK, BL)
    wmcat = np.ascontiguousarray(wmcat.reshape(C, 2 * O * C))
    # bias for block b lives at rows 96-127, column b
    b = np.zeros((128, 3), np.float32)
    bsrc = np.asarray(bias, np.float32).ravel()
    for o in range(O):
        b[96 + o % 32, o // 32] = bsrc[o]
    in_maps = []
    for core in range(N_CORES):
        n, half = core // 2, core % 2
        h0 = half * HS
        xs = np.ascontiguousarray(
            xpad[n, :, h0:h0 + 18, :].reshape(C, 18 * 34))
        in_maps.append({"xs": xs, "wmcat": wmcat, "bias": b})
    return in_maps


def run_sharded(x, weight, mask, bias, **run_kwargs):
    """Run on the 8 NeuronCores; returns (y_full, BassKernelResults)."""
    nc = _get_nc()
    in_maps = _prep_shards(x, weight, mask, bias)
    res = bass_utils.run_bass_kernel_spmd(
        nc, in_maps, core_ids=list(range(N_CORES)), **run_kwargs)
    n_img = np.asarray(x).shape[0]
    y = np.empty((n_img, O, 32, 32), dtype=np.float32)
    for core in range(N_CORES):
        n, half = core // 2, core % 2
        h0 = half * HS
        y[n, :, h0:h0 + HS, :] = res.results[core]["y"].reshape(O, HS, W)
    return y, res


def kernel(x, weight, mask, bias):
    y, _ = run_sharded(x, weight, mask, bias)
    return y



# revision 5
# speedup vs baseline: 1.6492x; 1.6492x over previous
"""Trainium2 Bass kernel: masked-bilinear channel-mixing Conv2d (ring-pair form).

reference math (N=4, C=96, H=W=32, O=96, K=3, PAD=1):
    p = avgpool3x3(x, count_include_pad) -> [N, C, H, W] -> [N, L=1024, C]
    wm = weight * mask                              [O, C, C]
    y[n,l,o] = sum_{c,d} wm[o,c,d] p[n,l,c] p[n,l,d] + bias[o]

The mask has a zero diagonal, so only c!=d pairs contribute. Writing
d = (c+k) mod 96, every ordered pair has a unique ring offset k in 1..95,
and offsets k and 96-k cover the same unordered pairs. Folding:

    y[o,l] = sum_{k=1..48} sum_c WM2_k[c,o] * r_k[c,l]
    r_k[c,l]  = p[c,l] * p[(c+k)%96, l]
    WM2_k     = A_k*maskA_k + B_k*maskB_k   (B only for k<48)
    A_k[c,o]  = weight[o, c, (c+k)%96],  B_k[c,o] = weight[o, (c+k)%96, c]

vs. the two-pass einsum dataflow this cuts PE work ~6x (95 accumulating
[96x96]x[96x512] matmuls into one PSUM bank) and halves the elementwise
middle stage (48*512 products/lane instead of 96*512 twice).

Sharding: data-parallel over locations -> 8 cores, each takes half of one
image (16 rows = 512 locations), all 96 output channels.

Per-core pipeline:
  PE warmup burst (lifts the HAM clock gate 1.2->2.4GHz) during DMAs
  pooling (DVE, f32) -> pt16 [96(c), 512(l)] bf16 (x 1/9)
  pt16 staged to a DRAM scratch [144, 512] ([p; p[0:48]]); one 3D DMA per
    k-group materializes rot[c, k, l] = p[(c+k)%96, l] (DRAM APs are linear
    so the k dim can walk partitions; SBUF APs cannot)
  r-ops: DVE bf16 TT 2x, in0 = pd12 (p repeated), in1 = rot group
  weights arrive as 8 blocks [w|m] (bf16, host pre-gathered in ring-pair
    unit order A1,B1,...,A48,Z); DVE multiplies w*m per block
  GEMM: 95 matmuls accumulate y in one PSUM bank; ACT adds bias on the
    PSUM->SBUF copy; DMA out.
"""
import numpy as np

import concourse.bass as bass
import concourse.bacc as bacc
import concourse.mybir as mybir
from concourse import tile
from concourse import bass_utils

C = 96
O = 96
HS = 16           # rows per core shard
W = 32
L = HS * W        # 512 locations per core
N_CORES = 8
NK = 48           # ring offsets
NBLK = 8          # weight DMA blocks (12 units each)
UPB = 12          # units per block
BLKW = UPB * O    # weight cols per block half (1152)
ROT_GROUPS = (4, 8, 12, 12, 12)   # k-group sizes for rot DMA / r ops
WARMUP_MMS = 12
F32 = mybir.dt.float32
BF16 = mybir.dt.bfloat16


def _build_kernel(nc: bass.Bass):
    xs_d = nc.dram_tensor("xs", [C, 18 * 34], F32, kind="ExternalInput")
    wcat_d = nc.dram_tensor("wcat", [C, NBLK * 2 * BLKW], BF16,
                            kind="ExternalInput")
    b_d = nc.dram_tensor("bias", [O, 1], F32, kind="ExternalInput")
    y_d = nc.dram_tensor("y", [O, L], F32, kind="ExternalOutput")

    with tile.TileContext(nc) as tc:
        with (
            tc.tile_pool(name="const", bufs=1) as cpool,
            tc.tile_pool(name="dram", bufs=1, space="DRAM") as dpool,
            tc.tile_pool(name="wps", bufs=1, space="PSUM") as wpsum,
            tc.tile_pool(name="yps", bufs=1, space="PSUM") as ypsum,
        ):
            # ---- PE warmup ASAP (garbage matmuls on a scratch bank) ----
            warm16 = cpool.tile([C, L], BF16)
            nc.vector.memset(warm16[:], 0.0)
            wps = wpsum.tile([C, L], F32)
            for _ in range(WARMUP_MMS):
                nc.tensor.matmul(wps[:], warm16[:, 0:C], warm16[:],
                                 start=True, stop=True, skip_group_check=True)

            # ---- input DMAs ----
            xs = cpool.tile([C, 18 * 34], F32)
            bias = cpool.tile([O, 1], F32)
            nc.sync.dma_start(xs[:], xs_d.ap())
            nc.sync.dma_start(bias[:], b_d.ap())
            wcat = cpool.tile([C, NBLK * 2 * BLKW], BF16)
            for g in range(NBLK):
                base = g * 2 * BLKW
                nc.scalar.dma_start(wcat[:, base:base + 2 * BLKW],
                                    wcat_d.ap()[:, base:base + 2 * BLKW])

            # ---- pooling: horizontal then vertical 3-tap box sums ----
            s1 = cpool.tile([C, 18 * 32], F32)
            s2 = cpool.tile([C, 18 * 32], F32)
            pt_raw = cpool.tile([C, L], F32)
            pt2 = cpool.tile([C, L], F32)
            pt16 = cpool.tile([C, L], BF16)
            x3 = xs[:].rearrange("c (h w) -> c h w", h=18)
            s1v = s1[:].rearrange("c (h w) -> c h w", h=18)
            s2v = s2[:].rearrange("c (h w) -> c h w", h=18)
            nc.vector.tensor_add(s1v, x3[:, :, 0:32], x3[:, :, 1:33])
            nc.vector.tensor_add(s2v, s1v, x3[:, :, 2:34])
            ptv = pt_raw[:].rearrange("c (h w) -> c h w", h=HS)
            pt2v = pt2[:].rearrange("c (h w) -> c h w", h=HS)
            nc.vector.tensor_add(pt2v, s2v[:, 0:16, :], s2v[:, 1:17, :])
            nc.vector.tensor_add(ptv, pt2v, s2v[:, 2:18, :])
            # p = boxsum/9; p enters the quadratic form twice -> 1/81 total
            nc.vector.tensor_scalar_mul(pt16[:], pt_raw[:], 1.0 / 9.0)

            # pd12 = p repeated 12x along free (in0 for every r op)
            pd12 = cpool.tile([C, 12 * L], BF16)
            nc.vector.tensor_copy(
                pd12[:].rearrange("c (r l) -> c r l", r=12),
                pt16[:].unsqueeze(1).broadcast_to((C, 12, L)))

            # ---- stage p to DRAM as [p; p[0:48]] for rotation DMAs ----
            pp = dpool.tile([C + NK, L], BF16)
            nc.sync.dma_start(pp[0:C, :], pt16[:])
            nc.sync.dma_start(pp[C:C + NK, :], pt16[0:NK, :])

            # ---- rot tiles + r ops + wm2 mults + GEMM, pipelined ----
            rot = cpool.tile([C, NK * L], BF16)
            r = cpool.tile([C, NK * L], BF16)
            wm2 = cpool.tile([C, NBLK * BLKW], BF16)
            y_ps = ypsum.tile([O, L], F32)
            y_sb = cpool.tile([O, L], F32)

            rot3 = rot[:].rearrange("c (j l) -> c j l", j=NK)
            pp_t = pp[:].tensor

            def rot_dma(k0, g, eng):
                # rot[c, k0-1+j, l] = pp[c + k0 + j, l],  j in [0, g)
                src = bass.AP(tensor=pp_t, offset=k0 * L,
                              ap=[[L, C], [L, g], [1, L]])
                eng.dma_start(rot3[:, k0 - 1:k0 - 1 + g, :], src)

            def wm2_block(g):
                base = g * 2 * BLKW
                nc.vector.tensor_mul(
                    wm2[:, g * BLKW:(g + 1) * BLKW],
                    wcat[:, base:base + BLKW],
                    wcat[:, base + BLKW:base + 2 * BLKW])

            # unit u (0..94): k = u//2+1, A side if u even else B side
            # (u==94 is A48; the Z padding unit 95 is never used)
            def unit_lhsT(u):
                g, uloc = u // UPB, u % UPB
                col = g * BLKW + uloc * O
                return wm2[:, col:col + O]

            mm_emitted = 0

            def emit_mms(k_hi):
                # emit GEMM matmuls for all units whose k <= k_hi
                nonlocal mm_emitted
                while mm_emitted < 95 and mm_emitted // 2 + 1 <= k_hi:
                    u = mm_emitted
                    k = u // 2 + 1
                    nc.tensor.matmul(
                        y_ps[:], unit_lhsT(u), r[:, (k - 1) * L:k * L],
                        start=(u == 0), stop=(u == 94))
                    mm_emitted += 1

            # interleave DVE work in expected-readiness order
            wm2_block(0)
            wm2_block(1)
            k0 = 1
            for gi, gsz in enumerate(ROT_GROUPS):
                rot_dma(k0, gsz, nc.sync)
                k0 += gsz
            k0 = 1
            wm2_done = 2
            for gi, gsz in enumerate(ROT_GROUPS):
                nc.vector.tensor_mul(
                    r[:, (k0 - 1) * L:(k0 - 1 + gsz) * L],
                    pd12[:, 0:gsz * L],
                    rot[:, (k0 - 1) * L:(k0 - 1 + gsz) * L])
                k0 += gsz
                # weight blocks needed up to k0-1: block g covers k<=6(g+1)
                while wm2_done * 6 < min(k0 - 1, NK) and wm2_done < NBLK:
                    wm2_block(wm2_done)
                    wm2_done += 1
                emit_mms(k0 - 1)
            while wm2_done < NBLK:
                wm2_block(wm2_done)
                wm2_done += 1
            emit_mms(NK)

            # ---- bias + output ----
            nc.scalar.activation(y_sb[:], y_ps[:],
                                 mybir.ActivationFunctionType.Identity,
                                 bias=bias[:])
            nc.sync.dma_start(y_d.ap(), y_sb[:])

    return nc


_NC_CACHE = {}


def _get_nc():
    if "nc" not in _NC_CACHE:
        nc = bacc.Bacc("TRN2", target_bir_lowering=False, debug=False,
                       enable_asserts=False)
        _build_kernel(nc)
        nc.compile()
        _NC_CACHE["nc"] = nc
    return _NC_CACHE["nc"]


def _to_bf16(a):
    import ml_dtypes
    return np.asarray(a, dtype=ml_dtypes.bfloat16)


def _prep_shards(x, weight, mask, bias):
    xpad = np.pad(np.asarray(x, np.float32), ((0, 0), (0, 0), (1, 1), (1, 1)))
    w = np.asarray(weight, np.float32)
    m = np.asarray(mask, np.float32)
    cs = np.arange(C)
    # units: A1,B1,A2,B2,...,A47,B47,A48,Z  -> [96 units][c][o]
    uw = np.zeros((2 * NK, C, O), np.float32)
    um = np.zeros((2 * NK, C, O), np.float32)
    for k in range(1, NK + 1):
        d = (cs + k) % C
        uw[2 * (k - 1)] = w[:, cs, d].T
        um[2 * (k - 1)] = m[:, cs, d].T
        if k < NK:
            uw[2 * (k - 1) + 1] = w[:, d, cs].T
            um[2 * (k - 1) + 1] = m[:, d, cs].T
    # pack blocks: [w(12 units x 96) | m(12 units x 96)] per block
    wcat = np.empty((C, NBLK, 2, UPB, O), np.float32)
    for g in range(NBLK):
        for ul in range(UPB):
            wcat[:, g, 0, ul, :] = uw[g * UPB + ul]
            wcat[:, g, 1, ul, :] = um[g * UPB + ul]
    wcat16 = _to_bf16(np.ascontiguousarray(
        wcat.reshape(C, NBLK * 2 * BLKW)))
    b = np.asarray(bias, np.float32).reshape(O, 1)
    in_maps = []
    for core in range(N_CORES):
        n, half = core // 2, core % 2
        h0 = half * HS
        xs = np.ascontiguousarray(
            xpad[n, :, h0:h0 + 18, :].reshape(C, 18 * 34))
        in_maps.append({"xs": xs, "wcat": wcat16, "bias": b})
    return in_maps


def run_sharded(x, weight, mask, bias, **run_kwargs):
    """Run on the 8 NeuronCores; returns (y_full, BassKernelResults)."""
    nc = _get_nc()
    in_maps = _prep_shards(x, weight, mask, bias)
    res = bass_utils.run_bass_kernel_spmd(
        nc, in_maps, core_ids=list(range(N_CORES)), **run_kwargs)
    n_img = np.asarray(x).shape[0]
    y = np.empty((n_img, O, 32, 32), dtype=np.float32)
    for core in range(N_CORES):
        n, half = core // 2, core % 2
        h0 = half * HS
        y[n, :, h0:h0 + HS, :] = res.results[core]["y"].reshape(O, HS, W)
    return y, res


def kernel(x, weight, mask, bias):
    y, _ = run_sharded(x, weight, mask, bias)
    return y
